# revision 3
# baseline (speedup 1.0000x reference)
"""Trainium2 Bass kernel for CentroidEdgeConvNet (2-layer mean-aggregation GNN).

Reference computation (N=100000 nodes, DEG=16, F=H=128, C=40):
    h1 = relu(mean_k feats[nbr[i,k]] @ W0 + b0)            # [N, H]
    out = log_softmax(mean_k h1[nbr2[i,k]] @ W1 + b1)      # [N, C],  nbr2 = neighbors[ids]

Sharding: nodes data-parallel over 8 cores (12500/core, padded to 12544 = 98
tiles of 128).  feats + weights replicated.

v2 vs the v1 baseline:
  * ONE multi-offset indirect DMA per node-tile ([128, 16] offset AP gathering
    all 2048 neighbor rows) instead of 16 single-offset DMAs -- the v1
    bottleneck was ~1us of SWDGE descriptor-generation overhead per DMA op.
  * phase 1 computes z = h1 @ (W1/16) + b1/16 (40 wide) immediately, so the
    exchanged table and the phase-2 gathers are 160B rows instead of 512B
    (z-table trick: the phase-2 matmul commutes with the neighbor mean).
  * The 1/16 mean scaling and b1 are folded into W0/W1/b1 on the host.

Host-side index prep (int32 gather index arrays, one per core) keeps the
device program identical across cores (pure SPMD + one collective).
"""

import numpy as np

import concourse.bacc as bacc
import concourse.bass as bass
import concourse.mybir as mybir
import concourse.tile as tile
from concourse.bass import IndirectOffsetOnAxis
from concourse.bass_utils import run_bass_kernel_spmd
from concourse.masks import make_identity

# Problem constants (hardcoded per harness contract)
N_NODES = 100000
DEG = 16
F = 128
H = 128
C = 40
NCORES = 8
P = 128

NSHARD = N_NODES // NCORES          # 12500
TILES = (NSHARD + P - 1) // P       # 98
NP_ROWS = TILES * P                 # 12544 padded shard rows
TBL_ROWS = NP_ROWS * NCORES         # 100352 rows in the all-gathered z table

F32 = mybir.dt.float32
I32 = mybir.dt.int32


def build_program(n_nodes=N_NODES, tiles=TILES, ncores=NCORES, ag_stripes=1,
                  phases="full", gbufs=4):
    """Build the SPMD Bass program (identical on all cores)."""
    np_rows = tiles * P
    tbl_rows = np_rows * ncores
    assert tiles % ag_stripes == 0
    stripe_tiles = tiles // ag_stripes

    nc = bacc.Bacc(
        "TRN2", target_bir_lowering=False, debug=False, num_devices=ncores
    )

    feats_t = nc.dram_tensor("feats", [n_nodes, F], F32, kind="ExternalInput")
    w0_t = nc.dram_tensor("w0", [F, H], F32, kind="ExternalInput")
    b0_t = nc.dram_tensor("b0", [H, 1], F32, kind="ExternalInput")
    w1_t = nc.dram_tensor("w1", [H, C], F32, kind="ExternalInput")
    b1_t = nc.dram_tensor("b1", [C, 1], F32, kind="ExternalInput")
    idx1_t = nc.dram_tensor("idx1", [P, tiles * DEG], I32, kind="ExternalInput")
    idx2_t = nc.dram_tensor("idx2", [P, tiles * DEG], I32, kind="ExternalInput")
    out_t = nc.dram_tensor("out", [P, tiles * C], F32, kind="ExternalOutput")

    AF = mybir.ActivationFunctionType
    ALU = mybir.AluOpType

    with tile.TileContext(nc) as tc:
        with (
            tc.tile_pool(name="const", bufs=1) as cpool,
            tc.tile_pool(name="gath", bufs=gbufs) as gpool,
            tc.tile_pool(name="work", bufs=3) as wpool,
            tc.tile_pool(name="small", bufs=8) as spool,
            tc.tile_pool(name="outp", bufs=1) as opool,
            tc.tile_pool(name="ps", bufs=2, space="PSUM") as pspool,
            tc.tile_pool(name="dram", bufs=1, space="DRAM") as dpool,
        ):
            # --- constants / parameters ---
            w0_sb = cpool.tile([F, H], F32, name="w0_sb")
            nc.sync.dma_start(w0_sb[:], w0_t.ap())
            w1_sb = cpool.tile([H, C], F32, name="w1_sb")
            nc.sync.dma_start(w1_sb[:], w1_t.ap())
            b0_sb = cpool.tile([H, 1], F32, name="b0_sb")
            nc.sync.dma_start(b0_sb[:], b0_t.ap())
            b1_sb = cpool.tile([C, 1], F32, name="b1_sb")
            nc.sync.dma_start(b1_sb[:], b1_t.ap())
            ident = cpool.tile([P, P], F32, name="ident")
            make_identity(nc, ident[:])

            idx1_sb = cpool.tile([P, tiles * DEG], I32, name="idx1_sb")
            nc.sync.dma_start(idx1_sb[:], idx1_t.ap())
            idx2_sb = cpool.tile([P, tiles * DEG], I32, name="idx2_sb")
            nc.sync.dma_start(idx2_sb[:], idx2_t.ap())

            out_acc = opool.tile([P, tiles * C], F32, name="out_acc")

            z_shard = dpool.tile([np_rows, C], F32, name="z_shard")
            z_full = dpool.tile(
                [tbl_rows, C], F32, name="z_full", addr_space="Shared"
            )

            # --- phase 1: local z shard (h1 -> z fused) ---
            for t in range(tiles):
                g = gpool.tile([P, DEG, F], F32, name="g", tag="g")
                nc.gpsimd.indirect_dma_start(
                    out=g[:],
                    out_offset=None,
                    in_=feats_t.ap(),
                    in_offset=IndirectOffsetOnAxis(
                        ap=idx1_sb[:, t * DEG : (t + 1) * DEG], axis=0
                    ),
                )
                # sum over the 16 gathered neighbor rows (1/16 folded into W0)
                m1 = wpool.tile([P, F], F32, name="m1", tag="m")
                nc.vector.tensor_reduce(
                    out=m1[:],
                    in_=g.rearrange("p a b -> p b a"),
                    axis=mybir.AxisListType.X,
                    op=ALU.add,
                )
                m1t_p = pspool.tile([P, P], F32, name="m1t_p", tag="mt_p")
                nc.tensor.transpose(m1t_p[:], m1[:], ident[:])
                m1t = wpool.tile([P, P], F32, name="m1t", tag="mt")
                nc.scalar.copy(m1t[:], m1t_p[:])
                h1t_p = pspool.tile([H, P], F32, name="h1t_p", tag="mm_p")
                nc.tensor.matmul(
                    h1t_p[:], lhsT=w0_sb[:], rhs=m1t[:], start=True, stop=True
                )
                h1t = wpool.tile([H, P], F32, name="h1t", tag="h1t")
                nc.scalar.activation(h1t[:], h1t_p[:], AF.Relu, bias=b0_sb[:, 0:1])
                z_p = pspool.tile([C, P], F32, name="z_p", tag="z_p")
                nc.tensor.matmul(
                    z_p[:], lhsT=w1_sb[:], rhs=h1t[:], start=True, stop=True
                )
                zc = wpool.tile([C, P], F32, name="zc", tag="zc")
                nc.scalar.activation(zc[:], z_p[:], AF.Identity, bias=b1_sb[:, 0:1])
                zt_p = pspool.tile([P, C], F32, name="zt_p", tag="zt_p")
                nc.tensor.transpose(zt_p[:], zc[:], ident[:C, :C])
                zt = wpool.tile([P, C], F32, name="zt", tag="zt")
                nc.vector.tensor_copy(zt[:], zt_p[:])
                nc.sync.dma_start(z_shard[t * P : (t + 1) * P, :], zt[:])

            # --- exchange z shards (optionally striped for overlap) ---
            srows = stripe_tiles * P
            n_ags = 0 if phases == "p1noag" else ag_stripes
            for s in range(n_ags):
                nc.gpsimd.collective_compute(
                    "AllGather",
                    ALU.bypass,
                    replica_groups=[list(range(ncores))],
                    ins=[z_shard[s * srows : (s + 1) * srows, :].opt()],
                    outs=[
                        z_full[
                            s * srows * ncores : (s + 1) * srows * ncores, :
                        ].opt()
                    ],
                )

            # --- phase 2: gather z, mean, log_softmax ---
            # phases: p1 (skip phase 2), p2g (gathers only), p2r (+reduce),
            # full (everything)
            p2_tiles = 0 if phases in ("p1", "p1noag") else tiles
            for t in range(p2_tiles):
                g2 = gpool.tile([P, DEG, C], F32, name="g2", tag="g2")
                nc.gpsimd.indirect_dma_start(
                    out=g2[:],
                    out_offset=None,
                    in_=z_full[:],
                    in_offset=IndirectOffsetOnAxis(
                        ap=idx2_sb[:, t * DEG : (t + 1) * DEG], axis=0
                    ),
                )
                if phases == "p2g":
                    continue
                # h2 = sum_k z[nbr2[*,k]]  (mean + bias already folded into z)
                m2 = wpool.tile([P, C], F32, name="m2", tag="m2")
                nc.vector.tensor_reduce(
                    out=m2[:],
                    in_=g2.rearrange("p a b -> p b a"),
                    axis=mybir.AxisListType.X,
                    op=ALU.add,
                )
                if phases == "p2r":
                    continue
                # log_softmax over the C free elements per node-partition
                nmax = spool.tile([P, 1], F32, name="nmax", tag="nmax")
                nc.vector.tensor_reduce(
                    out=nmax[:],
                    in_=m2[:],
                    axis=mybir.AxisListType.X,
                    op=ALU.max,
                    negate=True,
                )
                e = wpool.tile([P, C], F32, name="e", tag="e")
                ssum = spool.tile([P, 1], F32, name="ssum", tag="ssum")
                nc.scalar.activation(
                    e[:], m2[:], AF.Exp, bias=nmax[:, 0:1], accum_out=ssum[:, 0:1]
                )
                lse = spool.tile([P, 1], F32, name="lse", tag="lse")
                nc.scalar.activation(lse[:], ssum[:], AF.Ln)
                # out = (x + (-max)) - lse, fused in one DVE op
                nc.vector.scalar_tensor_tensor(
                    out=out_acc[:, t * C : (t + 1) * C],
                    in0=m2[:],
                    scalar=nmax[:, 0:1],
                    in1=lse[:, 0:1].to_broadcast([P, C]),
                    op0=mybir.AluOpType.add,
                    op1=mybir.AluOpType.subtract,
                )

            if phases != "full":
                nc.vector.memset(out_acc[:, 0:C], 0.0)
            nc.sync.dma_start(out_t.ap(), out_acc[:])

    nc.compile()
    return nc


def make_host_inputs(feats, W0, b0, W1, b1, ids, neighbors, n_nodes=N_NODES,
                     tiles=TILES, ncores=NCORES, ag_stripes=1):
    """Build per-core input maps (index prep + weight folding on host)."""
    np_rows = tiles * P
    nshard = n_nodes // ncores
    assert tiles % ag_stripes == 0
    stripe_rows = (tiles // ag_stripes) * P

    neighbors = np.asarray(neighbors).astype(np.int64)
    ids = np.asarray(ids).astype(np.int64)
    nbr2 = neighbors[ids]  # [n_out, DEG] layer-2 neighbor sets

    feats = np.ascontiguousarray(np.asarray(feats, np.float32))
    w0s = np.ascontiguousarray(np.asarray(W0, np.float32) / DEG)
    w1s = np.ascontiguousarray(np.asarray(W1, np.float32) / DEG)
    b0c = np.ascontiguousarray(np.asarray(b0, np.float32).reshape(H, 1))
    b1c = np.ascontiguousarray(np.asarray(b1, np.float32).reshape(C, 1) / DEG)

    # map node id -> row in the all-gathered (padded, possibly striped) table
    def table_row(j):
        owner = j // nshard
        local = j - owner * nshard
        stripe = local // stripe_rows
        within = local - stripe * stripe_rows
        return (stripe * ncores + owner) * stripe_rows + within

    in_maps = []
    for c in range(ncores):
        base = c * nshard
        # phase-1 node ids for this core's padded shard, clamped for pad slots
        node = np.minimum(base + np.arange(np_rows), n_nodes - 1)
        i1 = neighbors[node]                       # [np_rows, DEG]
        i1 = i1.reshape(tiles, P, DEG).transpose(1, 0, 2).reshape(P, tiles * DEG)
        out_row = np.minimum(base + np.arange(np_rows), n_nodes - 1)
        i2 = table_row(nbr2[out_row])              # [np_rows, DEG]
        i2 = i2.reshape(tiles, P, DEG).transpose(1, 0, 2).reshape(P, tiles * DEG)
        in_maps.append(
            {
                "feats": feats,
                "w0": w0s,
                "b0": b0c,
                "w1": w1s,
                "b1": b1c,
                "idx1": np.ascontiguousarray(i1.astype(np.int32)),
                "idx2": np.ascontiguousarray(i2.astype(np.int32)),
            }
        )
    return in_maps


def unshard_output(results, n_nodes=N_NODES, tiles=TILES, ncores=NCORES):
    """results: list of per-core {"out": [P, tiles*C]} -> full [n_nodes, C]."""
    nshard = n_nodes // ncores
    parts = []
    for c in range(ncores):
        o = np.asarray(results[c]["out"]).reshape(P, tiles, C)
        o = o.transpose(1, 0, 2).reshape(tiles * P, C)[:nshard]
        parts.append(o)
    return np.ascontiguousarray(np.concatenate(parts, axis=0).astype(np.float32))


_NC_CACHE = {}


def _get_program(key=(N_NODES, TILES, NCORES, 1)):
    if key not in _NC_CACHE:
        _NC_CACHE[key] = build_program(*key)
    return _NC_CACHE[key]


def kernel(**inputs):
    nc = _get_program()
    in_maps = make_host_inputs(
        inputs["feats"], inputs["W0"], inputs["b0"], inputs["W1"], inputs["b1"],
        inputs["ids"], inputs["neighbors"],
    )
    res = run_bass_kernel_spmd(nc, in_maps, core_ids=list(range(NCORES)))
    return unshard_output(res.results)


# revision 5
# speedup vs baseline: 1.0213x; 1.0213x over previous
"""Trainium2 Bass kernel for CentroidEdgeConvNet (2-layer mean-aggregation GNN).

Reference computation (N=100000 nodes, DEG=16, F=H=128, C=40):
    h1 = relu(mean_k feats[nbr[i,k]] @ W0 + b0)            # [N, H]
    out = log_softmax(mean_k h1[nbr2[i,k]] @ W1 + b1)      # [N, C],  nbr2 = neighbors[ids]

Sharding: nodes data-parallel over 8 cores (12500/core, padded to 12544 = 98
tiles of 128).  feats + weights replicated.

v2 vs the v1 baseline:
  * ONE multi-offset indirect DMA per node-tile ([128, 16] offset AP gathering
    all 2048 neighbor rows) instead of 16 single-offset DMAs -- the v1
    bottleneck was ~1us of SWDGE descriptor-generation overhead per DMA op.
  * phase 1 computes z = h1 @ (W1/16) + b1/16 (40 wide) immediately, so the
    exchanged table and the phase-2 gathers are 160B rows instead of 512B
    (z-table trick: the phase-2 matmul commutes with the neighbor mean).
  * The 1/16 mean scaling and b1 are folded into W0/W1/b1 on the host.

Host-side index prep (int32 gather index arrays, one per core) keeps the
device program identical across cores (pure SPMD + one collective).
"""

import numpy as np

import concourse.bacc as bacc
import concourse.bass as bass
import concourse.mybir as mybir
import concourse.tile as tile
from concourse.bass import IndirectOffsetOnAxis
from concourse.bass_utils import run_bass_kernel_spmd
from concourse.masks import make_identity

# Problem constants (hardcoded per harness contract)
N_NODES = 100000
DEG = 16
F = 128
H = 128
C = 40
NCORES = 8
P = 128

NSHARD = N_NODES // NCORES          # 12500
TILES = (NSHARD + P - 1) // P       # 98
NP_ROWS = TILES * P                 # 12544 padded shard rows
TBL_ROWS = NP_ROWS * NCORES         # 100352 rows in the all-gathered z table

F32 = mybir.dt.float32
I32 = mybir.dt.int32


def build_program(n_nodes=N_NODES, tiles=TILES, ncores=NCORES, ag_stripes=1,
                  phases="full", gbufs=4):
    """Build the SPMD Bass program (identical on all cores)."""
    np_rows = tiles * P
    tbl_rows = np_rows * ncores
    assert tiles % ag_stripes == 0
    stripe_tiles = tiles // ag_stripes

    nc = bacc.Bacc(
        "TRN2", target_bir_lowering=False, debug=False, num_devices=ncores
    )

    feats_t = nc.dram_tensor("feats", [n_nodes, F], F32, kind="ExternalInput")
    w0_t = nc.dram_tensor("w0", [F, H], F32, kind="ExternalInput")
    b0_t = nc.dram_tensor("b0", [H, 1], F32, kind="ExternalInput")
    w1_t = nc.dram_tensor("w1", [H, C], F32, kind="ExternalInput")
    b1_t = nc.dram_tensor("b1", [C, 1], F32, kind="ExternalInput")
    idx1_t = nc.dram_tensor("idx1", [P, tiles * DEG], I32, kind="ExternalInput")
    idx2_t = nc.dram_tensor("idx2", [P, tiles * DEG], I32, kind="ExternalInput")
    out_t = nc.dram_tensor("out", [P, tiles * C], F32, kind="ExternalOutput")

    AF = mybir.ActivationFunctionType
    ALU = mybir.AluOpType

    with tile.TileContext(nc) as tc:
        with (
            tc.tile_pool(name="const", bufs=1) as cpool,
            tc.tile_pool(name="gath", bufs=gbufs) as gpool,
            tc.tile_pool(name="work", bufs=3) as wpool,
            tc.tile_pool(name="small", bufs=8) as spool,
            tc.tile_pool(name="outp", bufs=1) as opool,
            tc.tile_pool(name="ps", bufs=2, space="PSUM") as pspool,
            tc.tile_pool(name="dram", bufs=1, space="DRAM") as dpool,
        ):
            # --- constants / parameters ---
            w0_sb = cpool.tile([F, H], F32, name="w0_sb")
            nc.sync.dma_start(w0_sb[:], w0_t.ap())
            w1_sb = cpool.tile([H, C], F32, name="w1_sb")
            nc.sync.dma_start(w1_sb[:], w1_t.ap())
            b0_sb = cpool.tile([H, 1], F32, name="b0_sb")
            nc.sync.dma_start(b0_sb[:], b0_t.ap())
            b1_sb = cpool.tile([C, 1], F32, name="b1_sb")
            nc.sync.dma_start(b1_sb[:], b1_t.ap())
            ident = cpool.tile([P, P], F32, name="ident")
            make_identity(nc, ident[:])

            idx1_sb = cpool.tile([P, tiles * DEG], I32, name="idx1_sb")
            nc.sync.dma_start(idx1_sb[:], idx1_t.ap())
            idx2_sb = cpool.tile([P, tiles * DEG], I32, name="idx2_sb")
            nc.sync.dma_start(idx2_sb[:], idx2_t.ap())

            out_acc = opool.tile([P, tiles * C], F32, name="out_acc")

            z_shard = dpool.tile([np_rows, C], F32, name="z_shard")
            z_full = dpool.tile(
                [tbl_rows, C], F32, name="z_full", addr_space="Shared"
            )

            # --- phase 1: local z shard (h1 -> z fused) ---
            for t in range(tiles):
                g = gpool.tile([P, DEG, F], F32, name="g", tag="g")
                # HW DynamicAP indirect DMA honors one offset per partition
                # -> 16 gathers of 128 rows each per node-tile
                for k in range(DEG):
                    nc.gpsimd.indirect_dma_start(
                        out=g[:, k, :],
                        out_offset=None,
                        in_=feats_t.ap(),
                        in_offset=IndirectOffsetOnAxis(
                            ap=idx1_sb[:, t * DEG + k : t * DEG + k + 1], axis=0
                        ),
                    )
                # sum over the 16 gathered neighbor rows (1/16 folded into W0)
                m1 = wpool.tile([P, F], F32, name="m1", tag="m")
                nc.vector.tensor_reduce(
                    out=m1[:],
                    in_=g.rearrange("p a b -> p b a"),
                    axis=mybir.AxisListType.X,
                    op=ALU.add,
                )
                m1t_p = pspool.tile([P, P], F32, name="m1t_p", tag="mt_p")
                nc.tensor.transpose(m1t_p[:], m1[:], ident[:])
                m1t = wpool.tile([P, P], F32, name="m1t", tag="mt")
                nc.scalar.copy(m1t[:], m1t_p[:])
                h1t_p = pspool.tile([H, P], F32, name="h1t_p", tag="mm_p")
                nc.tensor.matmul(
                    h1t_p[:], lhsT=w0_sb[:], rhs=m1t[:], start=True, stop=True
                )
                h1t = wpool.tile([H, P], F32, name="h1t", tag="h1t")
                nc.scalar.activation(h1t[:], h1t_p[:], AF.Relu, bias=b0_sb[:, 0:1])
                z_p = pspool.tile([C, P], F32, name="z_p", tag="z_p")
                nc.tensor.matmul(
                    z_p[:], lhsT=w1_sb[:], rhs=h1t[:], start=True, stop=True
                )
                zc = wpool.tile([C, P], F32, name="zc", tag="zc")
                nc.scalar.activation(zc[:], z_p[:], AF.Identity, bias=b1_sb[:, 0:1])
                zt_p = pspool.tile([P, C], F32, name="zt_p", tag="zt_p")
                nc.tensor.transpose(zt_p[:], zc[:], ident[:C, :C])
                zt = wpool.tile([P, C], F32, name="zt", tag="zt")
                nc.vector.tensor_copy(zt[:], zt_p[:])
                nc.sync.dma_start(z_shard[t * P : (t + 1) * P, :], zt[:])

            # --- exchange z shards (optionally striped for overlap) ---
            srows = stripe_tiles * P
            n_ags = 0 if phases == "p1noag" else ag_stripes
            for s in range(n_ags):
                nc.gpsimd.collective_compute(
                    "AllGather",
                    ALU.bypass,
                    replica_groups=[list(range(ncores))],
                    ins=[z_shard[s * srows : (s + 1) * srows, :].opt()],
                    outs=[
                        z_full[
                            s * srows * ncores : (s + 1) * srows * ncores, :
                        ].opt()
                    ],
                )

            # --- phase 2: gather z, mean, log_softmax ---
            # phases: p1 (skip phase 2), p2g (gathers only), p2r (+reduce),
            # full (everything)
            p2_tiles = 0 if phases in ("p1", "p1noag") else tiles
            for t in range(p2_tiles):
                g2 = gpool.tile([P, DEG, C], F32, name="g2", tag="g2")
                for k in range(DEG):
                    nc.gpsimd.indirect_dma_start(
                        out=g2[:, k, :],
                        out_offset=None,
                        in_=z_full[:],
                        in_offset=IndirectOffsetOnAxis(
                            ap=idx2_sb[:, t * DEG + k : t * DEG + k + 1], axis=0
                        ),
                    )
                if phases == "p2g":
                    continue
                # h2 = sum_k z[nbr2[*,k]]  (mean + bias already folded into z)
                m2 = wpool.tile([P, C], F32, name="m2", tag="m2")
                nc.vector.tensor_reduce(
                    out=m2[:],
                    in_=g2.rearrange("p a b -> p b a"),
                    axis=mybir.AxisListType.X,
                    op=ALU.add,
                )
                if phases == "p2r":
                    continue
                # log_softmax over the C free elements per node-partition
                nmax = spool.tile([P, 1], F32, name="nmax", tag="nmax")
                nc.vector.tensor_reduce(
                    out=nmax[:],
                    in_=m2[:],
                    axis=mybir.AxisListType.X,
                    op=ALU.max,
                    negate=True,
                )
                e = wpool.tile([P, C], F32, name="e", tag="e")
                ssum = spool.tile([P, 1], F32, name="ssum", tag="ssum")
                nc.scalar.activation(
                    e[:], m2[:], AF.Exp, bias=nmax[:, 0:1], accum_out=ssum[:, 0:1]
                )
                lse = spool.tile([P, 1], F32, name="lse", tag="lse")
                nc.scalar.activation(lse[:], ssum[:], AF.Ln)
                # out = (x + (-max)) - lse, fused in one DVE op
                nc.vector.scalar_tensor_tensor(
                    out=out_acc[:, t * C : (t + 1) * C],
                    in0=m2[:],
                    scalar=nmax[:, 0:1],
                    in1=lse[:, 0:1].to_broadcast([P, C]),
                    op0=mybir.AluOpType.add,
                    op1=mybir.AluOpType.subtract,
                )

            if phases != "full":
                nc.vector.memset(out_acc[:, 0:C], 0.0)
            nc.sync.dma_start(out_t.ap(), out_acc[:])

    nc.compile()
    return nc


def make_host_inputs(feats, W0, b0, W1, b1, ids, neighbors, n_nodes=N_NODES,
                     tiles=TILES, ncores=NCORES, ag_stripes=1):
    """Build per-core input maps (index prep + weight folding on host)."""
    np_rows = tiles * P
    nshard = n_nodes // ncores
    assert tiles % ag_stripes == 0
    stripe_rows = (tiles // ag_stripes) * P

    neighbors = np.asarray(neighbors).astype(np.int64)
    ids = np.asarray(ids).astype(np.int64)
    nbr2 = neighbors[ids]  # [n_out, DEG] layer-2 neighbor sets

    feats = np.ascontiguousarray(np.asarray(feats, np.float32))
    w0s = np.ascontiguousarray(np.asarray(W0, np.float32) / DEG)
    w1s = np.ascontiguousarray(np.asarray(W1, np.float32) / DEG)
    b0c = np.ascontiguousarray(np.asarray(b0, np.float32).reshape(H, 1))
    b1c = np.ascontiguousarray(np.asarray(b1, np.float32).reshape(C, 1) / DEG)

    # map node id -> row in the all-gathered (padded, possibly striped) table
    def table_row(j):
        owner = j // nshard
        local = j - owner * nshard
        stripe = local // stripe_rows
        within = local - stripe * stripe_rows
        return (stripe * ncores + owner) * stripe_rows + within

    in_maps = []
    for c in range(ncores):
        base = c * nshard
        # phase-1 node ids for this core's padded shard, clamped for pad slots
        node = np.minimum(base + np.arange(np_rows), n_nodes - 1)
        i1 = neighbors[node]                       # [np_rows, DEG]
        i1 = i1.reshape(tiles, P, DEG).transpose(1, 0, 2).reshape(P, tiles * DEG)
        out_row = np.minimum(base + np.arange(np_rows), n_nodes - 1)
        i2 = table_row(nbr2[out_row])              # [np_rows, DEG]
        i2 = i2.reshape(tiles, P, DEG).transpose(1, 0, 2).reshape(P, tiles * DEG)
        in_maps.append(
            {
                "feats": feats,
                "w0": w0s,
                "b0": b0c,
                "w1": w1s,
                "b1": b1c,
                "idx1": np.ascontiguousarray(i1.astype(np.int32)),
                "idx2": np.ascontiguousarray(i2.astype(np.int32)),
            }
        )
    return in_maps


def unshard_output(results, n_nodes=N_NODES, tiles=TILES, ncores=NCORES):
    """results: list of per-core {"out": [P, tiles*C]} -> full [n_nodes, C]."""
    nshard = n_nodes // ncores
    parts = []
    for c in range(ncores):
        o = np.asarray(results[c]["out"]).reshape(P, tiles, C)
        o = o.transpose(1, 0, 2).reshape(tiles * P, C)[:nshard]
        parts.append(o)
    return np.ascontiguousarray(np.concatenate(parts, axis=0).astype(np.float32))


_NC_CACHE = {}


def _get_program(key=(N_NODES, TILES, NCORES, 1)):
    if key not in _NC_CACHE:
        _NC_CACHE[key] = build_program(*key)
    return _NC_CACHE[key]


def kernel(**inputs):
    nc = _get_program()
    in_maps = make_host_inputs(
        inputs["feats"], inputs["W0"], inputs["b0"], inputs["W1"], inputs["b1"],
        inputs["ids"], inputs["neighbors"],
    )
    res = run_bass_kernel_spmd(nc, in_maps, core_ids=list(range(NCORES)))
    return unshard_output(res.results)


# revision 18
# speedup vs baseline: 1.1838x; 1.1591x over previous
"""Trainium2 Bass kernel for CentroidEdgeConvNet (2-layer mean-aggregation GNN).

Reference computation (N=100000 nodes, DEG=16, F=H=128, C=40):
    h1 = relu(mean_k feats[nbr[i,k]] @ W0 + b0)            # [N, H]
    out = log_softmax(mean_k h1[nbr2[i,k]] @ W1 + b1)      # [N, C],  nbr2 = neighbors[ids]

Sharding: nodes data-parallel over 8 cores (12500/core, padded to 12544 = 98
tiles of 128).  feats + weights replicated.

v3: neighbor aggregation via batched int16 DMA ops instead of per-128-row
indirect DMAs (the v1/v2 bottleneck: ~3k Pool-engine SWDGE ops at ~1us+ each).
Per phase, the 200704 edges are partitioned host-side into (neighbor-slot k x
source-bucket) chunks -- source rows bucketed into <=25088-row table slices so
in-slice indices fit dma_gather's int16, and a single k per chunk guarantees
collision-free destinations for dma_scatter_add (its CCE add loses updates on
duplicate indices within one op).  Each chunk is one dma_gather (rows ->
SBUF, position-preserving) + one dma_scatter_add (SBUF -> f32 accumulator in
DRAM).  Chunks are padded to a fixed 3584 indices with dummy-valid entries
(src row 0 of the slice, unique trash destination rows) so num_idxs is
compile-time constant.  Phase 1 accumulates sum_k feats[nbr] (512B rows);
a per-tile matmul pipeline then produces z = h1 @ (W1/16) + b1/16 padded to
64 f32 (256B rows); an AllGather shares z; phase 2 accumulates sum_k z[nbr2]
the same way and finishes with log_softmax.  The 1/16 mean scaling and b1
are folded into W0/W1/b1 on the host.
"""

import numpy as np

import concourse.bacc as bacc
import concourse.bass as bass
import concourse.mybir as mybir
import concourse.tile as tile
from concourse.bass_utils import run_bass_kernel_spmd
from concourse.masks import make_identity

# Problem constants (hardcoded per harness contract)
N_NODES = 100000
DEG = 16
F = 128
H = 128
C = 40
NCORES = 8
P = 128

NSHARD = N_NODES // NCORES          # 12500
TILES = (NSHARD + P - 1) // P       # 98
NP_ROWS = TILES * P                 # 12544 padded shard rows
TBL_ROWS = NP_ROWS * NCORES         # 100352 rows in the all-gathered z table

ZPAD = 64                           # z rows padded to 64 f32 = 256B for dma_gather
NBUCK = 4                           # source-row buckets (table slices, int16-safe)
B1 = (N_NODES + NBUCK - 1) // NBUCK      # 25000 feats rows per slice
B2 = TBL_ROWS // NBUCK                   # 25088 z rows per slice
SUBS = 4                            # sub-chunks per (k, bucket) list
CHUNK = 3584 // SUBS                # fixed idx count per DMA op (896)
TRASH = CHUNK                       # trash accumulator rows for dummy dests
ACC_ROWS = NP_ROWS + TRASH
NCHUNK = DEG * NBUCK * SUBS         # DMA op pairs per phase
ICOLS = CHUNK // 16                 # idx tile free dim

F32 = mybir.dt.float32
I16 = mybir.dt.int16

QUEUES = 4  # SWDGE queues (parallel Q7 descriptor-gen) and parallel accums


def build_program(tiles=TILES, ncores=NCORES, phases="full", gbufs=3):
    """Build the SPMD Bass program (identical on all cores)."""
    np_rows = tiles * P
    tbl_rows = np_rows * ncores

    nc = bacc.Bacc(
        "TRN2", target_bir_lowering=False, debug=False, num_devices=ncores,
        num_swdge_queues=QUEUES,
    )

    feats_t = nc.dram_tensor("feats", [N_NODES, F], F32, kind="ExternalInput")
    w0_t = nc.dram_tensor("w0", [F, H], F32, kind="ExternalInput")
    b0_t = nc.dram_tensor("b0", [H, 1], F32, kind="ExternalInput")
    w1_t = nc.dram_tensor("w1", [H, C], F32, kind="ExternalInput")
    b1_t = nc.dram_tensor("b1", [C, 1], F32, kind="ExternalInput")
    # per-chunk gather/scatter index tiles (int16, 16-partition wrapped)
    i1g_t = nc.dram_tensor("i1g", [P, NCHUNK * ICOLS], I16, kind="ExternalInput")
    i1s_t = nc.dram_tensor("i1s", [P, NCHUNK * ICOLS], I16, kind="ExternalInput")
    i2g_t = nc.dram_tensor("i2g", [P, NCHUNK * ICOLS], I16, kind="ExternalInput")
    i2s_t = nc.dram_tensor("i2s", [P, NCHUNK * ICOLS], I16, kind="ExternalInput")
    out_t = nc.dram_tensor("out", [P, tiles * C], F32, kind="ExternalOutput")

    AF = mybir.ActivationFunctionType
    ALU = mybir.AluOpType

    with tile.TileContext(nc) as tc:
        with (
            tc.tile_pool(name="const", bufs=1) as cpool,
            tc.tile_pool(name="gath", bufs=gbufs) as gpool,
            tc.tile_pool(name="idx", bufs=1) as ipool,
            tc.tile_pool(name="work", bufs=3) as wpool,
            tc.tile_pool(name="small", bufs=8) as spool,
            tc.tile_pool(name="outp", bufs=1) as opool,
            tc.tile_pool(name="ps", bufs=2, space="PSUM") as pspool,
            tc.tile_pool(name="dram", bufs=1, space="DRAM") as dpool,
        ):
            # --- constants / parameters ---
            w0_sb = cpool.tile([F, H], F32, name="w0_sb")
            nc.sync.dma_start(w0_sb[:], w0_t.ap())
            w1_sb = cpool.tile([H, C], F32, name="w1_sb")
            nc.sync.dma_start(w1_sb[:], w1_t.ap())
            b0_sb = cpool.tile([H, 1], F32, name="b0_sb")
            nc.sync.dma_start(b0_sb[:], b0_t.ap())
            b1_sb = cpool.tile([C, 1], F32, name="b1_sb")
            nc.sync.dma_start(b1_sb[:], b1_t.ap())
            ident = cpool.tile([P, P], F32, name="ident")
            make_identity(nc, ident[:])

            out_acc = opool.tile([P, tiles * C], F32, name="out_acc")

            msum = [
                dpool.tile([ACC_ROWS, F], F32, name=f"msum{q}")
                for q in range(QUEUES)
            ]
            h2acc = [
                dpool.tile([ACC_ROWS, ZPAD], F32, name=f"h2acc{q}")
                for q in range(QUEUES)
            ]
            z_shard = dpool.tile([np_rows, ZPAD], F32, name="z_shard")
            z_full = dpool.tile(
                [tbl_rows, ZPAD], F32, name="z_full", addr_space="Shared"
            )

            # --- zero the accumulators (real rows only; trash rows never read)
            ZR = 512
            zrow = cpool.tile([P, (ZR // P) * F], F32, name="zrow")
            nc.vector.memset(zrow[:], 0.0)
            zrow_f = zrow.rearrange("p (a b) -> p a b", b=F)
            for acc in msum:
                for r in range(0, NP_ROWS, ZR):
                    n = min(ZR, NP_ROWS - r)
                    nc.sync.dma_start(
                        acc[r : r + n, :].rearrange("(a p) b -> p a b", p=P),
                        zrow_f[:, : n // P, :],
                    )
            zrow_z = zrow.rearrange("p (a b) -> p a b", b=ZPAD)
            for acc in h2acc:
                for r in range(0, NP_ROWS, ZR * 2):
                    n = min(ZR * 2, NP_ROWS - r)
                    nc.sync.dma_start(
                        acc[r : r + n, :].rearrange("(a p) b -> p a b", p=P),
                        zrow_z[:, : n // P, :],
                    )

            def agg_phase(src_ap, gidx_t, sidx_t, accs, elem, bucket_rows, label):
                """NCHUNK chunks of gather(src rows) + scatter-add(acc rows).

                Chunk ci runs on SWDGE queue ci%QUEUES and accumulates into
                accs[ci%QUEUES] -- separate accumulators keep the scatter
                WAW chains short and the 4 Q7 desc-gen pairs busy.
                """
                gi = ipool.tile([P, NCHUNK * ICOLS], I16, name=f"gi{label}",
                                tag="gi")
                nc.sync.dma_start(gi[:], gidx_t.ap())
                si = ipool.tile([P, NCHUNK * ICOLS], I16, name=f"si{label}",
                                tag="si")
                nc.sync.dma_start(si[:], sidx_t.ap())
                for ci in range(NCHUNK):
                    b = (ci // SUBS) % NBUCK
                    q = ci % QUEUES
                    g = gpool.tile([P, CHUNK // P, elem], F32, name=f"g{label}",
                                   tag="g")
                    nc.gpsimd.dma_gather(
                        out_ap=g[:],
                        in_ap=src_ap[b * bucket_rows :, :],
                        idxs_ap=gi[:, ci * ICOLS : (ci + 1) * ICOLS],
                        num_idxs=CHUNK,
                        num_idxs_reg=CHUNK,
                        elem_size=elem,
                        queue_num=q,
                    )
                    nc.gpsimd.dma_scatter_add(
                        out_ap=accs[q][:],
                        in_ap=g[:],
                        idxs_ap=si[:, ci * ICOLS : (ci + 1) * ICOLS],
                        num_idxs=CHUNK,
                        num_idxs_reg=CHUNK,
                        elem_size=elem,
                        queue_num=q,
                    )

            # --- phase 1: accumulate sum_k feats[nbr[i,k]] into msum ---
            agg_phase(feats_t.ap(), i1g_t, i1s_t, msum, F, B1, "1")

            # --- phase 1b: per-tile matmuls msum -> z (padded to ZPAD) ---
            for t in range(tiles):
                mp = [
                    wpool.tile([P, F], F32, name=f"mp{q}", tag=f"mp{q}")
                    for q in range(QUEUES)
                ]
                for q in range(QUEUES):
                    nc.sync.dma_start(mp[q][:], msum[q][t * P : (t + 1) * P, :])
                m1 = wpool.tile([P, F], F32, name="m1", tag="m")
                nc.vector.tensor_tensor(
                    out=m1[:], in0=mp[0][:], in1=mp[1][:],
                    op=mybir.AluOpType.add,
                )
                nc.vector.tensor_tensor(
                    out=mp[2][:], in0=mp[2][:], in1=mp[3][:],
                    op=mybir.AluOpType.add,
                )
                nc.vector.tensor_tensor(
                    out=m1[:], in0=m1[:], in1=mp[2][:],
                    op=mybir.AluOpType.add,
                )
                m1t_p = pspool.tile([P, P], F32, name="m1t_p", tag="mt_p")
                nc.tensor.transpose(m1t_p[:], m1[:], ident[:])
                m1t = wpool.tile([P, P], F32, name="m1t", tag="mt")
                nc.scalar.copy(m1t[:], m1t_p[:])
                h1t_p = pspool.tile([H, P], F32, name="h1t_p", tag="mm_p")
                nc.tensor.matmul(
                    h1t_p[:], lhsT=w0_sb[:], rhs=m1t[:], start=True, stop=True
                )
                h1t = wpool.tile([H, P], F32, name="h1t", tag="h1t")
                nc.scalar.activation(h1t[:], h1t_p[:], AF.Relu, bias=b0_sb[:, 0:1])
                z_p = pspool.tile([C, P], F32, name="z_p", tag="z_p")
                nc.tensor.matmul(
                    z_p[:], lhsT=w1_sb[:], rhs=h1t[:], start=True, stop=True
                )
                zc = wpool.tile([C, P], F32, name="zc", tag="zc")
                nc.scalar.activation(zc[:], z_p[:], AF.Identity, bias=b1_sb[:, 0:1])
                zt_p = pspool.tile([P, C], F32, name="zt_p", tag="zt_p")
                nc.tensor.transpose(zt_p[:], zc[:], ident[:C, :C])
                zt = wpool.tile([P, ZPAD], F32, name="zt", tag="zt")
                nc.vector.memset(zt[:, C:], 0.0)
                nc.vector.tensor_copy(zt[:, :C], zt_p[:])
                nc.sync.dma_start(z_shard[t * P : (t + 1) * P, :], zt[:])

            # --- exchange z shards ---
            if phases != "p1noag":
                nc.gpsimd.collective_compute(
                    "AllGather",
                    ALU.bypass,
                    replica_groups=[list(range(ncores))],
                    ins=[z_shard[:].opt()],
                    outs=[z_full[:].opt()],
                )

            # --- phase 2: accumulate sum_k z[nbr2[i,k]] into h2acc ---
            if phases not in ("p1", "p1noag"):
                agg_phase(z_full[:], i2g_t, i2s_t, h2acc, ZPAD, B2, "2")

            # --- phase 2b: per-tile log_softmax ---
            p2_tiles = 0 if phases in ("p1", "p1noag", "p2g") else tiles
            for t in range(p2_tiles):
                hp = [
                    wpool.tile([P, ZPAD], F32, name=f"hp{q}", tag=f"hp{q}")
                    for q in range(QUEUES)
                ]
                for q in range(QUEUES):
                    nc.sync.dma_start(hp[q][:], h2acc[q][t * P : (t + 1) * P, :])
                m2 = wpool.tile([P, ZPAD], F32, name="m2", tag="m2")
                nc.vector.tensor_tensor(
                    out=m2[:], in0=hp[0][:], in1=hp[1][:],
                    op=mybir.AluOpType.add,
                )
                nc.vector.tensor_tensor(
                    out=hp[2][:], in0=hp[2][:], in1=hp[3][:],
                    op=mybir.AluOpType.add,
                )
                nc.vector.tensor_tensor(
                    out=m2[:], in0=m2[:], in1=hp[2][:],
                    op=mybir.AluOpType.add,
                )
                nmax = spool.tile([P, 1], F32, name="nmax", tag="nmax")
                nc.vector.tensor_reduce(
                    out=nmax[:],
                    in_=m2[:, :C],
                    axis=mybir.AxisListType.X,
                    op=ALU.max,
                    negate=True,
                )
                e = wpool.tile([P, C], F32, name="e", tag="e")
                ssum = spool.tile([P, 1], F32, name="ssum", tag="ssum")
                nc.scalar.activation(
                    e[:], m2[:, :C], AF.Exp, bias=nmax[:, 0:1],
                    accum_out=ssum[:, 0:1]
                )
                lse = spool.tile([P, 1], F32, name="lse", tag="lse")
                nc.scalar.activation(lse[:], ssum[:], AF.Ln)
                nc.vector.scalar_tensor_tensor(
                    out=out_acc[:, t * C : (t + 1) * C],
                    in0=m2[:, :C],
                    scalar=nmax[:, 0:1],
                    in1=lse[:, 0:1].to_broadcast([P, C]),
                    op0=mybir.AluOpType.add,
                    op1=mybir.AluOpType.subtract,
                )

            if phases != "full":
                nc.vector.memset(out_acc[:, 0:C], 0.0)
            nc.sync.dma_start(out_t.ap(), out_acc[:])

    nc.compile()
    return nc


def _wrap16(vals, n):
    """int16 idx layout: value i at partition i%16, col i//16, replicated x8."""
    t = np.full((16, n // 16), -1, np.int16)
    flat = np.asarray(vals, np.int64)
    t[np.arange(len(flat)) % 16, np.arange(len(flat)) // 16] = flat.astype(np.int16)
    return np.tile(t, (8, 1))


def _chunk_lists(nbr_local, bucket_rows, n_buckets=NBUCK, chunk=CHUNK, subs=SUBS):
    """Edge lists per (k, bucket, sub) chunk: (gather local idx, scatter dest).

    nbr_local: [np_rows, DEG] source rows (phase table space).
    Returns two [P, NCHUNK*ICOLS] int16 arrays.  Each (k, bucket) list is
    split into `subs` fixed-size pieces padded with dummy-valid entries
    (src row 0, unique trash dests) so every DMA op has exactly `chunk`
    valid indices and collision-free destinations.
    """
    np_rows = nbr_local.shape[0]
    dest = np.arange(np_rows, dtype=np.int64)
    gcols, scols = [], []
    for k in range(DEG):
        src_k = nbr_local[:, k]
        for b in range(n_buckets):
            m = (src_k >= b * bucket_rows) & (src_k < (b + 1) * bucket_rows)
            g = src_k[m] - b * bucket_rows
            s = dest[m]
            total = chunk * subs
            npad = total - len(g)
            assert npad > 0, f"chunk overflow: {len(g)} > {total}"
            g = np.concatenate([g, np.zeros(npad, np.int64)])
            s = np.concatenate([s, np.full(npad, -1, np.int64)])
            for p in range(subs):
                gp = g[p * chunk : (p + 1) * chunk].copy()
                sp = s[p * chunk : (p + 1) * chunk].copy()
                pad = sp < 0
                sp[pad] = np_rows + np.arange(chunk, dtype=np.int64)[pad]
                gcols.append(_wrap16(gp, chunk))
                scols.append(_wrap16(sp, chunk))
    return (
        np.ascontiguousarray(np.concatenate(gcols, axis=1)),
        np.ascontiguousarray(np.concatenate(scols, axis=1)),
    )


def make_host_inputs(feats, W0, b0, W1, b1, ids, neighbors,
                     tiles=TILES, ncores=NCORES):
    """Build per-core input maps (index prep + weight folding on host)."""
    np_rows = tiles * P
    nshard = N_NODES // ncores

    neighbors = np.asarray(neighbors).astype(np.int64)
    ids = np.asarray(ids).astype(np.int64)
    nbr2 = neighbors[ids]  # [n_out, DEG] layer-2 neighbor sets

    feats = np.ascontiguousarray(np.asarray(feats, np.float32))
    w0s = np.ascontiguousarray(np.asarray(W0, np.float32) / DEG)
    w1s = np.ascontiguousarray(np.asarray(W1, np.float32) / DEG)
    b0c = np.ascontiguousarray(np.asarray(b0, np.float32).reshape(H, 1))
    b1c = np.ascontiguousarray(np.asarray(b1, np.float32).reshape(C, 1) / DEG)

    # node id -> row in the all-gathered z table
    def table_row(j):
        owner = j // nshard
        return owner * np_rows + (j - owner * nshard)

    in_maps = []
    for c in range(ncores):
        base = c * nshard
        node = np.minimum(base + np.arange(np_rows), N_NODES - 1)
        i1g, i1s = _chunk_lists(neighbors[node], B1)
        i2g, i2s = _chunk_lists(table_row(nbr2[node]), B2)
        in_maps.append(
            {
                "feats": feats,
                "w0": w0s,
                "b0": b0c,
                "w1": w1s,
                "b1": b1c,
                "i1g": i1g,
                "i1s": i1s,
                "i2g": i2g,
                "i2s": i2s,
            }
        )
    return in_maps


def io_signature():
    """(name, shape, dtype, kind) of the program I/O -- for perf.build_null."""
    return [
        ("feats", [N_NODES, F], F32, "ExternalInput"),
        ("w0", [F, H], F32, "ExternalInput"),
        ("b0", [H, 1], F32, "ExternalInput"),
        ("w1", [H, C], F32, "ExternalInput"),
        ("b1", [C, 1], F32, "ExternalInput"),
        ("i1g", [P, NCHUNK * ICOLS], I16, "ExternalInput"),
        ("i1s", [P, NCHUNK * ICOLS], I16, "ExternalInput"),
        ("i2g", [P, NCHUNK * ICOLS], I16, "ExternalInput"),
        ("i2s", [P, NCHUNK * ICOLS], I16, "ExternalInput"),
        ("out", [P, TILES * C], F32, "ExternalOutput"),
    ]


def unshard_output(results, tiles=TILES, ncores=NCORES):
    """results: list of per-core {"out": [P, tiles*C]} -> full [N, C]."""
    nshard = N_NODES // ncores
    parts = []
    for c in range(ncores):
        o = np.asarray(results[c]["out"]).reshape(P, tiles, C)
        o = o.transpose(1, 0, 2).reshape(tiles * P, C)[:nshard]
        parts.append(o)
    return np.ascontiguousarray(np.concatenate(parts, axis=0).astype(np.float32))


_NC_CACHE = {}


def _get_program(key=(TILES, NCORES)):
    if key not in _NC_CACHE:
        _NC_CACHE[key] = build_program(*key)
    return _NC_CACHE[key]


def kernel(**inputs):
    nc = _get_program()
    in_maps = make_host_inputs(
        inputs["feats"], inputs["W0"], inputs["b0"], inputs["W1"], inputs["b1"],
        inputs["ids"], inputs["neighbors"],
    )
    res = run_bass_kernel_spmd(nc, in_maps, core_ids=list(range(NCORES)))
    return unshard_output(res.results)
